# revision 29
# baseline (speedup 1.0000x reference)
"""Distributed Llama-attention Bass kernel for 8 TRN2 NeuronCores.

Sharding: tensor-parallel over heads for QKV + attention (core c owns query
heads 4c..4c+3 and KV head c), then per-chunk AllGathers of the attention
outputs (bf16, 512KB/rank each) pipelined against later chunks, and a
column-shard of wo so each core produces a disjoint [2048, 512] column slice
of the final output (no all-reduce).
"""

import math
import sys

import numpy as np

sys.path.insert(0, "/opt/trn_rl_repo")

import ml_dtypes  # noqa: E402

import concourse.bass as bass  # noqa: E402
import concourse.mybir as mybir  # noqa: E402
import concourse.tile as tile  # noqa: E402
from concourse import bacc  # noqa: E402
from concourse.bass_utils import run_bass_kernel_spmd  # noqa: E402
from concourse.masks import make_identity  # noqa: E402

F32 = mybir.dt.float32
BF16 = mybir.dt.bfloat16
Alu = mybir.AluOpType
Act = mybir.ActivationFunctionType

NCORES = 8
S = 2048
D = 4096
H = 32
HKV = 8
HD = 128
NH = H // NCORES          # 4 local query heads
QCOLS = NH * HD           # 512 local q-proj cols
CHUNK = 512               # s-chunk size
NCHUNK = S // CHUNK       # 4
DC = D // 128             # 32 d-chunks
SCALE = 1.0 / math.sqrt(HD)

_CACHED = {}


def _build_graph():
    nc = bacc.Bacc(
        "TRN2",
        target_bir_lowering=False,
        debug=False,
        num_devices=NCORES,
    )

    hs_d = nc.dram_tensor("hs", [S, D], F32, kind="ExternalInput").ap()
    wq_d = nc.dram_tensor("wq", [D, QCOLS], F32, kind="ExternalInput").ap()
    wk_d = nc.dram_tensor("wk", [D, HD], F32, kind="ExternalInput").ap()
    wv_d = nc.dram_tensor("wv", [D, HD], F32, kind="ExternalInput").ap()
    wo_d = nc.dram_tensor("wo", [D, QCOLS], F32, kind="ExternalInput").ap()
    cos_d = nc.dram_tensor("cos", [HD, S], BF16, kind="ExternalInput").ap()
    sin_d = nc.dram_tensor("sin", [HD, S], BF16, kind="ExternalInput").ap()
    out_d = nc.dram_tensor("out", [S, QCOLS], F32, kind="ExternalOutput").ap()

    with tile.TileContext(nc) as tc:
        with (
            tc.tile_pool(name="persist", bufs=1) as pp,
            tc.tile_pool(name="stage", bufs=4) as stg,
            tc.tile_pool(name="hst", bufs=1) as hstp,
            tc.tile_pool(name="qtp", bufs=2) as qtp,
            tc.tile_pool(name="otp", bufs=2) as otp,
            tc.tile_pool(name="ep", bufs=4) as ep,
            tc.tile_pool(name="rt", bufs=2) as rtp,
            tc.tile_pool(name="ps_acc", bufs=4, space="PSUM") as ps_acc,
            tc.tile_pool(name="ps_r", bufs=1, space="PSUM") as ps_r,
            tc.tile_pool(name="ps_sc", bufs=3, space="PSUM") as ps_sc,
            tc.tile_pool(name="dram", bufs=1, space="DRAM") as dram,
        ):
            # ---------------- persistent SBUF tensors ----------------
            wq_bf = pp.tile([128, DC, QCOLS], BF16, tag="wq")
            wk_bf = pp.tile([128, DC, HD], BF16, tag="wk")
            wv_bf = pp.tile([128, DC, HD], BF16, tag="wv")
            wo_bf = pp.tile([128, DC, QCOLS], BF16, tag="wo")
            cos_sb = pp.tile([HD, S], BF16, tag="cos")
            sin_sb = pp.tile([HD, S], BF16, tag="sin")
            kT_bf = pp.tile([HD, S], BF16, tag="kt")
            v_bf = pp.tile([128, S // 128, HD], BF16, tag="v")
            ident = pp.tile([128, 128], BF16, tag="id")
            ones_col = pp.tile([128, 1], BF16, tag="onc")
            ones_row = pp.tile([1, 128], BF16, tag="onr")

            hs_bf_dram = dram.tile([S, D], BF16, tag="hsbf", name="hsbf")
            attn_in = [
                [
                    dram.tile(
                        [HD, CHUNK], BF16, tag=f"ain{j}_{h}", name=f"ain{j}_{h}"
                    )
                    for h in range(NH)
                ]
                for j in range(NCHUNK)
            ]
            attn_all = [
                [
                    dram.tile(
                        [NCORES * HD, CHUNK], BF16, tag=f"aall{j}_{h}",
                        addr_space="Shared", name=f"aall{j}_{h}",
                    )
                    for h in range(NH)
                ]
                for j in range(NCHUNK)
            ]

            # ---------------- constants (cheap, first) ----------------
            make_identity(nc, ident[:])
            nc.gpsimd.memset(ones_col[:], 1.0)
            nc.gpsimd.memset(ones_row[:], 1.0)
            nc.sync.dma_start(out=cos_sb[:], in_=cos_d)
            nc.sync.dma_start(out=sin_sb[:], in_=sin_d)

            # ---------------- phase helpers ----------------
            def load_weight(dst_bf, src_d, width, eng, eng2=None):
                # two d-chunks per DMA (512KB when width=512) for DMA efficiency
                sview = src_d.rearrange("(i p) w -> p i w", p=128)
                for i in range(0, DC, 2):
                    ws = stg.tile([128, 1024], F32, tag="stage")
                    wsv = ws[:].rearrange("p (i w) -> p i w", i=2)
                    nc.scalar.dma_start(
                        out=wsv[:, :, :width], in_=sview[:, i : i + 2, :]
                    )
                    e1 = eng if (eng2 is None or (i // 2) % 2 == 0) else eng2
                    e1(out=dst_bf[:, i, :], in_=wsv[:, 0, :width])
                    e1(out=dst_bf[:, i + 1, :], in_=wsv[:, 1, :width])

            def build_hsT(j, hsT):
                """DMA hs rows, convert to bf16, bounce via DRAM, then use the
                DMA transpose engine into hsT. All on the sync ring, with the
                transposes grouped per chunk to minimize xbar-mode flips."""
                s0 = j * CHUNK
                for dp in range(4):  # 4 column pieces of 1024
                    for ss in range(CHUNK // 128):
                        r0 = s0 + 128 * ss
                        hstg = stg.tile([128, 1024], F32, tag="stage")
                        nc.sync.dma_start(
                            out=hstg[:],
                            in_=hs_d[r0 : r0 + 128, 1024 * dp : 1024 * (dp + 1)],
                        )
                        hsb = stg.tile([128, 1024], BF16, tag="hsb", bufs=2)
                        nc.vector.tensor_copy(out=hsb[:], in_=hstg[:])
                        nc.sync.dma_start(
                            out=hs_bf_dram[
                                r0 : r0 + 128, 1024 * dp : 1024 * (dp + 1)
                            ],
                            in_=hsb[:],
                        )
                for i in range(DC):
                    nc.sync.dma_start(
                        out=hsT[:, i, :],
                        in_=hs_bf_dram[s0 : s0 + CHUNK, 128 * i : 128 * (i + 1)],
                        transpose=True,
                    )

            def rope(psrc, dst_ap, sl):
                t1 = rtp.tile([128, CHUNK], F32, tag="rt")
                t2 = rtp.tile([128, CHUNK], F32, tag="rt")
                nc.vector.tensor_tensor(
                    out=t1[0:64, :], in0=psrc[64:128, :], in1=sin_sb[0:64, sl], op=Alu.mult
                )
                nc.vector.tensor_tensor(
                    out=t1[64:128, :], in0=psrc[0:64, :], in1=sin_sb[64:128, sl], op=Alu.mult
                )
                nc.vector.tensor_tensor(
                    out=t2[:], in0=psrc[:], in1=cos_sb[:, sl], op=Alu.mult
                )
                nc.vector.tensor_tensor(out=dst_ap, in0=t1[:], in1=t2[:], op=Alu.add)

            def projections(j, hsT, qT):
                s0 = j * CHUNK
                sl = bass.ds(s0, CHUNK)
                for h in range(NH):
                    psq = ps_acc.tile([128, 512], F32, tag="acc")
                    for i in range(DC):
                        nc.tensor.matmul(
                            psq[:],
                            lhsT=wq_bf[:, i, HD * h : HD * (h + 1)],
                            rhs=hsT[:, i, :],
                            start=(i == 0),
                            stop=(i == DC - 1),
                        )
                    rope(psq[:], qT[:, h, :], sl)
                psk = ps_acc.tile([128, 512], F32, tag="acc")
                for i in range(DC):
                    nc.tensor.matmul(
                        psk[:],
                        lhsT=wk_bf[:, i, :],
                        rhs=hsT[:, i, :],
                        start=(i == 0),
                        stop=(i == DC - 1),
                    )
                rope(psk[:], kT_bf[:, sl], sl)
                # v computed as vT (stationary = wv reused 512-wide), then
                # PE-transposed back to [s, hd] for the AV matmuls: avoids 512
                # LDWEIGHTS-bound 128-wide matmuls.
                psvT = ps_acc.tile([128, 512], F32, tag="acc")
                for i in range(DC):
                    nc.tensor.matmul(
                        psvT[:],
                        lhsT=wv_bf[:, i, :],
                        rhs=hsT[:, i, :],
                        start=(i == 0),
                        stop=(i == DC - 1),
                    )
                vT_sb = ep.tile([128, 512], BF16, tag="vts", bufs=2)
                nc.scalar.copy(out=vT_sb[:], in_=psvT[:])
                psv2 = ps_acc.tile([128, 4, 128], BF16, tag="acc")
                for ss in range(CHUNK // 128):
                    nc.tensor.transpose(
                        psv2[:, ss, :],
                        vT_sb[:, 128 * ss : 128 * (ss + 1)],
                        ident[:],
                    )
                nc.any.tensor_copy(
                    out=v_bf[:, (CHUNK // 128) * j : (CHUNK // 128) * (j + 1), :],
                    in_=psv2[:],
                )

            def attention(j, qT):
                nk = 4 * (j + 1)  # causal: key tiles 0..nk-1

                def score_block(h, kcs):
                    es = []
                    for kc in kcs:
                        pss = ps_sc.tile([128, 512], F32, tag="sc")
                        nc.tensor.matmul(
                            pss[:],
                            lhsT=kT_bf[:, 128 * kc : 128 * (kc + 1)],
                            rhs=qT[:, h, :],
                            start=True,
                            stop=True,
                        )
                        e = ep.tile([128, CHUNK], BF16, tag="e", bufs=8)
                        nc.scalar.activation(
                            out=e[:], in_=pss[:], func=Act.Exp, scale=SCALE
                        )
                        if kc >= 4 * j:
                            # causal mask in-place on the idle gpsimd:
                            # keep where q_rel - k_rel - 128*(kc-4j) >= 0
                            nc.gpsimd.affine_select(
                                out=e[:],
                                in_=e[:],
                                compare_op=Alu.is_ge,
                                fill=0.0,
                                base=-128 * (kc - 4 * j),
                                channel_multiplier=-1,
                                pattern=[[1, CHUNK]],
                            )
                        es.append(e)
                    return es

                def av_block(pso, es, kcs):
                    for e, kc in zip(es, kcs):
                        nc.tensor.matmul(
                            pso[:],
                            lhsT=v_bf[:, kc, :],
                            rhs=e[:],
                            start=(kc == 0),
                            stop=(kc == nk - 1),
                        )

                def racc_block(racc, es, kcs):
                    for e, kc in zip(es, kcs):
                        if kc == 0:
                            nc.vector.tensor_copy(out=racc[:], in_=e[:])
                        else:
                            nc.vector.tensor_tensor(
                                out=racc[:], in0=racc[:], in1=e[:], op=Alu.add
                            )

                # two heads interleaved: one head's score/AV matmuls hide the
                # other head's exp+mask latency on ACT/gpsimd
                for hp in range(NH // 2):
                    h0, h1 = 2 * hp, 2 * hp + 1
                    pso0 = ps_acc.tile([128, 512], F32, tag="acc")
                    pso1 = ps_acc.tile([128, 512], F32, tag="acc")
                    racc0 = ep.tile([128, 512], F32, tag="racc", bufs=2)
                    racc1 = ep.tile([128, 512], F32, tag="racc2", bufs=2)
                    for kb in range(0, nk, 4):
                        kcs = list(range(kb, min(kb + 4, nk)))
                        es0 = score_block(h0, kcs)
                        es1 = score_block(h1, kcs)
                        av_block(pso0, es0, kcs)
                        av_block(pso1, es1, kcs)
                        racc_block(racc0, es0, kcs)
                        racc_block(racc1, es1, kcs)
                    pairs = [(h0, pso0, racc0), (h1, pso1, racc1)]
                    for h, pso, racc in pairs:
                        # partition reduce 128 -> 1 with a single ones-matmul
                        rbf = ep.tile([128, 512], BF16, tag="rbf", bufs=2)
                        nc.scalar.copy(out=rbf[:], in_=racc[:])
                        psr = ps_r.tile([1, 512], F32, tag="r")
                        nc.tensor.matmul(
                            psr[:], lhsT=ones_col[:], rhs=rbf[:],
                            start=True, stop=True,
                        )
                        rc = ep.tile([1, 512], BF16, tag="rc", bufs=2)
                        with nc.allow_low_precision(
                            reason="1/rowsum bcast; bf16 fine for softmax norm"
                        ):
                            nc.vector.reciprocal(rc[:], psr[:])
                        psb = ps_r.tile([128, 512], F32, tag="r")
                        nc.tensor.matmul(
                            psb[:], lhsT=ones_row[:], rhs=rc[:],
                            start=True, stop=True,
                        )
                        sbb = ep.tile([128, 512], F32, tag="os", bufs=3)
                        nc.scalar.copy(out=sbb[:], in_=psb[:])
                        ao = ep.tile([128, CHUNK], BF16, tag="ao", bufs=2)
                        nc.vector.tensor_tensor(
                            out=ao[:], in0=pso[:], in1=sbb[:], op=Alu.mult
                        )
                        nc.scalar.dma_start(out=attn_in[j][h][:, :], in_=ao[:])
                        nc.gpsimd.collective_compute(
                            "AllGather",
                            Alu.bypass,
                            replica_groups=[list(range(NCORES))],
                            ins=[attn_in[j][h].opt()],
                            outs=[attn_all[j][h].opt()],
                        )

            def oproj(j):
                aviews = [
                    attn_all[j][h][:].rearrange("(r p) s -> p r s", p=128)
                    for h in range(NH)
                ]
                for ss in range(CHUNK // 128):
                    ot = otp.tile([128, DC, 128], BF16, tag="ot")
                    otv = ot[:].rearrange("p (r h) f -> p r h f", h=NH)
                    for h in range(NH):
                        nc.scalar.dma_start(
                            out=otv[:, :, h, :],
                            in_=aviews[h][:, :, 128 * ss : 128 * (ss + 1)],
                        )
                    pso2 = ps_acc.tile([128, 512], F32, tag="acc")
                    order = [4 * r + h for h in range(NH) for r in range(NCORES)]
                    for n, g in enumerate(order):
                        nc.tensor.matmul(
                            pso2[:],
                            lhsT=ot[:, g, :],
                            rhs=wo_bf[:, g, :],
                            start=(n == 0),
                            stop=(n == DC - 1),
                        )
                    osb = ep.tile([128, 512], F32, tag="os", bufs=3)
                    nc.any.tensor_copy(out=osb[:], in_=pso2[:])
                    r0 = j * CHUNK + 128 * ss
                    nc.sync.dma_start(out=out_d[r0 : r0 + 128, :], in_=osb[:])

            # ---------------- schedule ----------------
            # hs chunk 0 first so the PE starts transposing immediately;
            # q/k/v weights next (needed by chunk-0 projections); wo deferred.
            # hs chunk 0 first (feeds PE transposes immediately), then the
            # weights in the order projections consume them; wo deferred.
            hsT0 = hstp.tile([128, DC, CHUNK], BF16, tag="hsT")
            build_hsT(0, hsT0)
            load_weight(wq_bf, wq_d, QCOLS, nc.scalar.copy, nc.vector.tensor_copy)
            load_weight(wk_bf, wk_d, HD, nc.vector.tensor_copy)
            load_weight(wv_bf, wv_d, HD, nc.vector.tensor_copy)

            qT0 = qtp.tile([HD, NH, CHUNK], BF16, tag="qT")
            projections(0, hsT0, qT0)
            attention(0, qT0)

            for j in range(1, NCHUNK):
                hsT = hstp.tile([128, DC, CHUNK], BF16, tag="hsT")
                build_hsT(j, hsT)
                qT = qtp.tile([HD, NH, CHUNK], BF16, tag="qT")
                projections(j, hsT, qT)
                if j == 1:
                    load_weight(wo_bf, wo_d, QCOLS, nc.vector.tensor_copy, nc.scalar.copy)
                attention(j, qT)
                oproj(j - 1)
            oproj(NCHUNK - 1)

    nc.finalize()
    return nc


def _get_graph():
    if "nc" not in _CACHED:
        _CACHED["nc"] = _build_graph()
    return _CACHED["nc"]


def _rope_tables(position_ids):
    pos = np.asarray(position_ids).reshape(-1).astype(np.float64)  # [S]
    inv_freq = 1.0 / (10000.0 ** (np.arange(0, HD, 2, dtype=np.float64) / HD))  # [64]
    freqs = pos[:, None] * inv_freq[None, :]  # [S, 64]
    emb = np.concatenate([freqs, freqs], axis=-1)  # [S, HD]
    cos_t = np.cos(emb).T.astype(np.float32)  # [HD, S]
    sin_t = np.sin(emb).T.astype(np.float32)
    sin_signed = sin_t.copy()
    sin_signed[: HD // 2] *= -1.0
    bf = ml_dtypes.bfloat16
    return (
        np.ascontiguousarray(cos_t.astype(bf)),
        np.ascontiguousarray(sin_signed.astype(bf)),
    )


def kernel(hidden_states, wq, wk, wv, wo, position_ids, _trace=False):
    hs = np.ascontiguousarray(np.asarray(hidden_states, np.float32).reshape(S, D))
    wq = np.asarray(wq, np.float32)
    wk = np.asarray(wk, np.float32)
    wv = np.asarray(wv, np.float32)
    wo = np.asarray(wo, np.float32)
    cos_t, sin_t = _rope_tables(position_ids)

    in_maps = []
    for c in range(NCORES):
        in_maps.append(
            {
                "hs": hs,
                "wq": np.ascontiguousarray(wq[:, QCOLS * c : QCOLS * (c + 1)]),
                "wk": np.ascontiguousarray(wk[:, HD * c : HD * (c + 1)]),
                "wv": np.ascontiguousarray(wv[:, HD * c : HD * (c + 1)]),
                "wo": np.ascontiguousarray(wo[:, QCOLS * c : QCOLS * (c + 1)]),
                "cos": cos_t,
                "sin": sin_t,
            }
        )

    nc = _get_graph()
    res = run_bass_kernel_spmd(
        nc, in_maps, core_ids=list(range(NCORES)), trace=_trace
    )
    outs = [np.asarray(res.results[c]["out"]) for c in range(NCORES)]
    full = np.concatenate(outs, axis=1).reshape(1, S, D).astype(np.float32)
    if _trace:
        kernel.last_results = res
    return full


# revision 30
# speedup vs baseline: 1.3845x; 1.3845x over previous
"""Distributed Llama-attention Bass kernel for 8 TRN2 NeuronCores.

Sharding: tensor-parallel over heads for QKV + attention (core c owns query
heads 4c..4c+3 and KV head c), then per-chunk AllGathers of the attention
outputs (bf16, 512KB/rank each) pipelined against later chunks, and a
column-shard of wo so each core produces a disjoint [2048, 512] column slice
of the final output (no all-reduce).
"""

import math
import sys

import numpy as np

sys.path.insert(0, "/opt/trn_rl_repo")

import ml_dtypes  # noqa: E402

import concourse.bass as bass  # noqa: E402
import concourse.mybir as mybir  # noqa: E402
import concourse.tile as tile  # noqa: E402
from concourse import bacc  # noqa: E402
from concourse.bass_utils import run_bass_kernel_spmd  # noqa: E402
from concourse.masks import make_identity  # noqa: E402

F32 = mybir.dt.float32
BF16 = mybir.dt.bfloat16
Alu = mybir.AluOpType
Act = mybir.ActivationFunctionType

NCORES = 8
S = 2048
D = 4096
H = 32
HKV = 8
HD = 128
NH = H // NCORES          # 4 local query heads
QCOLS = NH * HD           # 512 local q-proj cols
CHUNK = 512               # s-chunk size
NCHUNK = S // CHUNK       # 4
DC = D // 128             # 32 d-chunks
SCALE = 1.0 / math.sqrt(HD)

_CACHED = {}


def _build_graph():
    nc = bacc.Bacc(
        "TRN2",
        target_bir_lowering=False,
        debug=False,
        num_devices=NCORES,
    )

    hs_d = nc.dram_tensor("hs", [S, D], F32, kind="ExternalInput").ap()
    wq_d = nc.dram_tensor("wq", [D, QCOLS], F32, kind="ExternalInput").ap()
    wk_d = nc.dram_tensor("wk", [D, HD], F32, kind="ExternalInput").ap()
    wv_d = nc.dram_tensor("wv", [D, HD], F32, kind="ExternalInput").ap()
    wo_d = nc.dram_tensor("wo", [D, QCOLS], F32, kind="ExternalInput").ap()
    cos_d = nc.dram_tensor("cos", [HD, S], BF16, kind="ExternalInput").ap()
    sin_d = nc.dram_tensor("sin", [HD, S], BF16, kind="ExternalInput").ap()
    out_d = nc.dram_tensor("out", [S, QCOLS], F32, kind="ExternalOutput").ap()

    with tile.TileContext(nc) as tc:
        with (
            tc.tile_pool(name="persist", bufs=1) as pp,
            tc.tile_pool(name="stage", bufs=4) as stg,
            tc.tile_pool(name="hst", bufs=1) as hstp,
            tc.tile_pool(name="qtp", bufs=2) as qtp,
            tc.tile_pool(name="otp", bufs=2) as otp,
            tc.tile_pool(name="ep", bufs=4) as ep,
            tc.tile_pool(name="rt", bufs=2) as rtp,
            tc.tile_pool(name="ps_acc", bufs=3, space="PSUM") as ps_acc,
            tc.tile_pool(name="ps_r", bufs=1, space="PSUM") as ps_r,
            tc.tile_pool(name="ps_sc", bufs=2, space="PSUM") as ps_sc,
            tc.tile_pool(name="ps_tr", bufs=2, space="PSUM") as ps_tr,
            tc.tile_pool(name="dram", bufs=1, space="DRAM") as dram,
        ):
            # ---------------- persistent SBUF tensors ----------------
            wq_bf = pp.tile([128, DC, QCOLS], BF16, tag="wq")
            wk_bf = pp.tile([128, DC, HD], BF16, tag="wk")
            wv_bf = pp.tile([128, DC, HD], BF16, tag="wv")
            wo_bf = pp.tile([128, DC, QCOLS], BF16, tag="wo")
            cos_sb = pp.tile([HD, S], BF16, tag="cos")
            sin_sb = pp.tile([HD, S], BF16, tag="sin")
            kT_bf = pp.tile([HD, S], BF16, tag="kt")
            v_bf = pp.tile([128, S // 128, HD], BF16, tag="v")
            ident = pp.tile([128, 128], BF16, tag="id")
            ones_col = pp.tile([128, 1], BF16, tag="onc")
            ones_row = pp.tile([1, 128], BF16, tag="onr")

            attn_in = [
                [
                    dram.tile(
                        [HD, CHUNK], BF16, tag=f"ain{j}_{h}", name=f"ain{j}_{h}"
                    )
                    for h in range(NH)
                ]
                for j in range(NCHUNK)
            ]
            attn_all = [
                [
                    dram.tile(
                        [NCORES * HD, CHUNK], BF16, tag=f"aall{j}_{h}",
                        addr_space="Shared", name=f"aall{j}_{h}",
                    )
                    for h in range(NH)
                ]
                for j in range(NCHUNK)
            ]

            # ---------------- constants (cheap, first) ----------------
            make_identity(nc, ident[:])
            nc.gpsimd.memset(ones_col[:], 1.0)
            nc.gpsimd.memset(ones_row[:], 1.0)
            nc.sync.dma_start(out=cos_sb[:], in_=cos_d)
            nc.sync.dma_start(out=sin_sb[:], in_=sin_d)

            # ---------------- phase helpers ----------------
            def load_weight(dst_bf, src_d, width, eng, eng2=None):
                # two d-chunks per DMA (512KB when width=512) for DMA efficiency
                sview = src_d.rearrange("(i p) w -> p i w", p=128)
                for i in range(0, DC, 2):
                    ws = stg.tile([128, 1024], F32, tag="stage")
                    wsv = ws[:].rearrange("p (i w) -> p i w", i=2)
                    nc.scalar.dma_start(
                        out=wsv[:, :, :width], in_=sview[:, i : i + 2, :]
                    )
                    e1 = eng if (eng2 is None or (i // 2) % 2 == 0) else eng2
                    e1(out=dst_bf[:, i, :], in_=wsv[:, 0, :width])
                    e1(out=dst_bf[:, i + 1, :], in_=wsv[:, 1, :width])

            def build_hsT(j, hsT):
                """DMA hs rows, convert to bf16, transpose with PE into hsT."""
                s0 = j * CHUNK
                for dp in range(4):  # 4 column pieces of 1024
                    for ss in range(CHUNK // 128):
                        r0 = s0 + 128 * ss
                        hstg = stg.tile([128, 1024], F32, tag="stage")
                        nc.sync.dma_start(
                            out=hstg[:],
                            in_=hs_d[r0 : r0 + 128, 1024 * dp : 1024 * (dp + 1)],
                        )
                        hsb = stg.tile([128, 1024], BF16, tag="hsb", bufs=2)
                        nc.vector.tensor_copy(out=hsb[:], in_=hstg[:])
                        for u in range(2):  # two groups of 4 transposes
                            tr = ps_tr.tile([128, 512], BF16, tag="tr")
                            for t in range(4):
                                kk = 4 * u + t
                                nc.tensor.transpose(
                                    tr[:, 128 * t : 128 * (t + 1)],
                                    hsb[:, 128 * kk : 128 * (kk + 1)],
                                    ident[:],
                                )
                            i0 = 8 * dp + 4 * u
                            nc.any.tensor_copy(
                                out=hsT[:, i0 : i0 + 4, 128 * ss : 128 * (ss + 1)],
                                in_=tr[:].rearrange("p (a b) -> p a b", a=4),
                            )

            def rope(psrc, dst_ap, sl):
                t1 = rtp.tile([128, CHUNK], F32, tag="rt")
                t2 = rtp.tile([128, CHUNK], F32, tag="rt")
                nc.vector.tensor_tensor(
                    out=t1[0:64, :], in0=psrc[64:128, :], in1=sin_sb[0:64, sl], op=Alu.mult
                )
                nc.vector.tensor_tensor(
                    out=t1[64:128, :], in0=psrc[0:64, :], in1=sin_sb[64:128, sl], op=Alu.mult
                )
                nc.vector.tensor_tensor(
                    out=t2[:], in0=psrc[:], in1=cos_sb[:, sl], op=Alu.mult
                )
                nc.vector.tensor_tensor(out=dst_ap, in0=t1[:], in1=t2[:], op=Alu.add)

            def projections(j, hsT, qT):
                s0 = j * CHUNK
                sl = bass.ds(s0, CHUNK)
                for h in range(NH):
                    psq = ps_acc.tile([128, 512], F32, tag="acc")
                    for i in range(DC):
                        nc.tensor.matmul(
                            psq[:],
                            lhsT=wq_bf[:, i, HD * h : HD * (h + 1)],
                            rhs=hsT[:, i, :],
                            start=(i == 0),
                            stop=(i == DC - 1),
                        )
                    rope(psq[:], qT[:, h, :], sl)
                psk = ps_acc.tile([128, 512], F32, tag="acc")
                for i in range(DC):
                    nc.tensor.matmul(
                        psk[:],
                        lhsT=wk_bf[:, i, :],
                        rhs=hsT[:, i, :],
                        start=(i == 0),
                        stop=(i == DC - 1),
                    )
                rope(psk[:], kT_bf[:, sl], sl)
                # v computed as vT (stationary = wv reused 512-wide), then
                # PE-transposed back to [s, hd] for the AV matmuls: avoids 512
                # LDWEIGHTS-bound 128-wide matmuls.
                psvT = ps_acc.tile([128, 512], F32, tag="acc")
                for i in range(DC):
                    nc.tensor.matmul(
                        psvT[:],
                        lhsT=wv_bf[:, i, :],
                        rhs=hsT[:, i, :],
                        start=(i == 0),
                        stop=(i == DC - 1),
                    )
                vT_sb = ep.tile([128, 512], BF16, tag="vts", bufs=2)
                nc.scalar.copy(out=vT_sb[:], in_=psvT[:])
                psv2 = ps_acc.tile([128, 4, 128], BF16, tag="acc")
                for ss in range(CHUNK // 128):
                    nc.tensor.transpose(
                        psv2[:, ss, :],
                        vT_sb[:, 128 * ss : 128 * (ss + 1)],
                        ident[:],
                    )
                nc.any.tensor_copy(
                    out=v_bf[:, (CHUNK // 128) * j : (CHUNK // 128) * (j + 1), :],
                    in_=psv2[:],
                )

            def attention(j, qT):
                nk = 4 * (j + 1)  # causal: key tiles 0..nk-1

                def score_block(h, kcs):
                    es = []
                    for kc in kcs:
                        pss = ps_sc.tile([128, 512], F32, tag="sc")
                        nc.tensor.matmul(
                            pss[:],
                            lhsT=kT_bf[:, 128 * kc : 128 * (kc + 1)],
                            rhs=qT[:, h, :],
                            start=True,
                            stop=True,
                        )
                        e = ep.tile([128, CHUNK], BF16, tag="e", bufs=8)
                        nc.scalar.activation(
                            out=e[:], in_=pss[:], func=Act.Exp, scale=SCALE
                        )
                        if kc >= 4 * j:
                            # causal mask in-place on the idle gpsimd:
                            # keep where q_rel - k_rel - 128*(kc-4j) >= 0
                            nc.gpsimd.affine_select(
                                out=e[:],
                                in_=e[:],
                                compare_op=Alu.is_ge,
                                fill=0.0,
                                base=-128 * (kc - 4 * j),
                                channel_multiplier=-1,
                                pattern=[[1, CHUNK]],
                            )
                        es.append(e)
                    return es

                def av_block(pso, es, kcs):
                    for e, kc in zip(es, kcs):
                        nc.tensor.matmul(
                            pso[:],
                            lhsT=v_bf[:, kc, :],
                            rhs=e[:],
                            start=(kc == 0),
                            stop=(kc == nk - 1),
                        )

                def racc_block(racc, es, kcs):
                    for e, kc in zip(es, kcs):
                        if kc == 0:
                            nc.vector.tensor_copy(out=racc[:], in_=e[:])
                        else:
                            nc.vector.tensor_tensor(
                                out=racc[:], in0=racc[:], in1=e[:], op=Alu.add
                            )

                # two heads interleaved: one head's score/AV matmuls hide the
                # other head's exp+mask latency on ACT/gpsimd
                for hp in range(NH // 2):
                    h0, h1 = 2 * hp, 2 * hp + 1
                    pso0 = ps_acc.tile([128, 512], F32, tag="acc")
                    pso1 = ps_acc.tile([128, 512], F32, tag="acc")
                    racc0 = ep.tile([128, 512], F32, tag="racc", bufs=2)
                    racc1 = ep.tile([128, 512], F32, tag="racc2", bufs=2)
                    for kb in range(0, nk, 4):
                        kcs = list(range(kb, min(kb + 4, nk)))
                        es0 = score_block(h0, kcs)
                        es1 = score_block(h1, kcs)
                        av_block(pso0, es0, kcs)
                        av_block(pso1, es1, kcs)
                        racc_block(racc0, es0, kcs)
                        racc_block(racc1, es1, kcs)
                    pairs = [(h0, pso0, racc0), (h1, pso1, racc1)]
                    for h, pso, racc in pairs:
                        # partition reduce 128 -> 1 with a single ones-matmul
                        rbf = ep.tile([128, 512], BF16, tag="rbf", bufs=2)
                        nc.scalar.copy(out=rbf[:], in_=racc[:])
                        psr = ps_r.tile([1, 512], F32, tag="r")
                        nc.tensor.matmul(
                            psr[:], lhsT=ones_col[:], rhs=rbf[:],
                            start=True, stop=True,
                        )
                        rc = ep.tile([1, 512], BF16, tag="rc", bufs=2)
                        with nc.allow_low_precision(
                            reason="1/rowsum bcast; bf16 fine for softmax norm"
                        ):
                            nc.vector.reciprocal(rc[:], psr[:])
                        psb = ps_r.tile([128, 512], F32, tag="r")
                        nc.tensor.matmul(
                            psb[:], lhsT=ones_row[:], rhs=rc[:],
                            start=True, stop=True,
                        )
                        sbb = ep.tile([128, 512], F32, tag="os", bufs=3)
                        nc.scalar.copy(out=sbb[:], in_=psb[:])
                        ao = ep.tile([128, CHUNK], BF16, tag="ao", bufs=2)
                        nc.vector.tensor_tensor(
                            out=ao[:], in0=pso[:], in1=sbb[:], op=Alu.mult
                        )
                        nc.sync.dma_start(out=attn_in[j][h][:, :], in_=ao[:])
                        nc.gpsimd.collective_compute(
                            "AllGather",
                            Alu.bypass,
                            replica_groups=[list(range(NCORES))],
                            ins=[attn_in[j][h].opt()],
                            outs=[attn_all[j][h].opt()],
                        )

            def oproj(j):
                aviews = [
                    attn_all[j][h][:].rearrange("(r p) s -> p r s", p=128)
                    for h in range(NH)
                ]
                for ss in range(CHUNK // 128):
                    ot = otp.tile([128, DC, 128], BF16, tag="ot")
                    otv = ot[:].rearrange("p (r h) f -> p r h f", h=NH)
                    for h in range(NH):
                        nc.scalar.dma_start(
                            out=otv[:, :, h, :],
                            in_=aviews[h][:, :, 128 * ss : 128 * (ss + 1)],
                        )
                    pso2 = ps_acc.tile([128, 512], F32, tag="acc")
                    order = [4 * r + h for h in range(NH) for r in range(NCORES)]
                    for n, g in enumerate(order):
                        nc.tensor.matmul(
                            pso2[:],
                            lhsT=ot[:, g, :],
                            rhs=wo_bf[:, g, :],
                            start=(n == 0),
                            stop=(n == DC - 1),
                        )
                    osb = ep.tile([128, 512], F32, tag="os", bufs=3)
                    nc.any.tensor_copy(out=osb[:], in_=pso2[:])
                    r0 = j * CHUNK + 128 * ss
                    nc.sync.dma_start(out=out_d[r0 : r0 + 128, :], in_=osb[:])

            # ---------------- schedule ----------------
            # hs chunk 0 first so the PE starts transposing immediately;
            # q/k/v weights next (needed by chunk-0 projections); wo deferred.
            # hs chunk 0 first (feeds PE transposes immediately), then the
            # weights in the order projections consume them; wo deferred.
            hsT0 = hstp.tile([128, DC, CHUNK], BF16, tag="hsT")
            build_hsT(0, hsT0)
            load_weight(wq_bf, wq_d, QCOLS, nc.scalar.copy, nc.vector.tensor_copy)
            load_weight(wk_bf, wk_d, HD, nc.vector.tensor_copy)
            load_weight(wv_bf, wv_d, HD, nc.vector.tensor_copy)

            qT0 = qtp.tile([HD, NH, CHUNK], BF16, tag="qT")
            projections(0, hsT0, qT0)
            attention(0, qT0)

            for j in range(1, NCHUNK):
                hsT = hstp.tile([128, DC, CHUNK], BF16, tag="hsT")
                build_hsT(j, hsT)
                qT = qtp.tile([HD, NH, CHUNK], BF16, tag="qT")
                projections(j, hsT, qT)
                if j == 1:
                    load_weight(wo_bf, wo_d, QCOLS, nc.vector.tensor_copy, nc.scalar.copy)
                attention(j, qT)
                oproj(j - 1)
            oproj(NCHUNK - 1)

    nc.finalize()
    return nc


def _get_graph():
    if "nc" not in _CACHED:
        _CACHED["nc"] = _build_graph()
    return _CACHED["nc"]


def _rope_tables(position_ids):
    pos = np.asarray(position_ids).reshape(-1).astype(np.float64)  # [S]
    inv_freq = 1.0 / (10000.0 ** (np.arange(0, HD, 2, dtype=np.float64) / HD))  # [64]
    freqs = pos[:, None] * inv_freq[None, :]  # [S, 64]
    emb = np.concatenate([freqs, freqs], axis=-1)  # [S, HD]
    cos_t = np.cos(emb).T.astype(np.float32)  # [HD, S]
    sin_t = np.sin(emb).T.astype(np.float32)
    sin_signed = sin_t.copy()
    sin_signed[: HD // 2] *= -1.0
    bf = ml_dtypes.bfloat16
    return (
        np.ascontiguousarray(cos_t.astype(bf)),
        np.ascontiguousarray(sin_signed.astype(bf)),
    )


def kernel(hidden_states, wq, wk, wv, wo, position_ids, _trace=False):
    hs = np.ascontiguousarray(np.asarray(hidden_states, np.float32).reshape(S, D))
    wq = np.asarray(wq, np.float32)
    wk = np.asarray(wk, np.float32)
    wv = np.asarray(wv, np.float32)
    wo = np.asarray(wo, np.float32)
    cos_t, sin_t = _rope_tables(position_ids)

    in_maps = []
    for c in range(NCORES):
        in_maps.append(
            {
                "hs": hs,
                "wq": np.ascontiguousarray(wq[:, QCOLS * c : QCOLS * (c + 1)]),
                "wk": np.ascontiguousarray(wk[:, HD * c : HD * (c + 1)]),
                "wv": np.ascontiguousarray(wv[:, HD * c : HD * (c + 1)]),
                "wo": np.ascontiguousarray(wo[:, QCOLS * c : QCOLS * (c + 1)]),
                "cos": cos_t,
                "sin": sin_t,
            }
        )

    nc = _get_graph()
    res = run_bass_kernel_spmd(
        nc, in_maps, core_ids=list(range(NCORES)), trace=_trace
    )
    outs = [np.asarray(res.results[c]["out"]) for c in range(NCORES)]
    full = np.concatenate(outs, axis=1).reshape(1, S, D).astype(np.float32)
    if _trace:
        kernel.last_results = res
    return full


# revision 31
# speedup vs baseline: 1.4381x; 1.0387x over previous
"""Distributed Llama-attention Bass kernel for 8 TRN2 NeuronCores.

Sharding: tensor-parallel over heads for QKV + attention (core c owns query
heads 4c..4c+3 and KV head c), then per-chunk AllGathers of the attention
outputs (bf16, 512KB/rank each) pipelined against later chunks, and a
column-shard of wo so each core produces a disjoint [2048, 512] column slice
of the final output (no all-reduce).
"""

import math
import sys

import numpy as np

sys.path.insert(0, "/opt/trn_rl_repo")

import ml_dtypes  # noqa: E402

import concourse.bass as bass  # noqa: E402
import concourse.mybir as mybir  # noqa: E402
import concourse.tile as tile  # noqa: E402
from concourse import bacc  # noqa: E402
from concourse.bass_utils import run_bass_kernel_spmd  # noqa: E402
from concourse.masks import make_identity  # noqa: E402

F32 = mybir.dt.float32
BF16 = mybir.dt.bfloat16
Alu = mybir.AluOpType
Act = mybir.ActivationFunctionType

NCORES = 8
S = 2048
D = 4096
H = 32
HKV = 8
HD = 128
NH = H // NCORES          # 4 local query heads
QCOLS = NH * HD           # 512 local q-proj cols
CHUNK = 512               # s-chunk size
NCHUNK = S // CHUNK       # 4
DC = D // 128             # 32 d-chunks
SCALE = 1.0 / math.sqrt(HD)

_CACHED = {}


def _build_graph():
    nc = bacc.Bacc(
        "TRN2",
        target_bir_lowering=False,
        debug=False,
        num_devices=NCORES,
    )

    hs_d = nc.dram_tensor("hs", [S, D], F32, kind="ExternalInput").ap()
    wq_d = nc.dram_tensor("wq", [D, QCOLS], F32, kind="ExternalInput").ap()
    wk_d = nc.dram_tensor("wk", [D, HD], F32, kind="ExternalInput").ap()
    wv_d = nc.dram_tensor("wv", [D, HD], F32, kind="ExternalInput").ap()
    wo_d = nc.dram_tensor("wo", [D, QCOLS], F32, kind="ExternalInput").ap()
    cos_d = nc.dram_tensor("cos", [HD, S], BF16, kind="ExternalInput").ap()
    sin_d = nc.dram_tensor("sin", [HD, S], BF16, kind="ExternalInput").ap()
    out_d = nc.dram_tensor("out", [S, QCOLS], F32, kind="ExternalOutput").ap()

    with tile.TileContext(nc) as tc:
        with (
            tc.tile_pool(name="persist", bufs=1) as pp,
            tc.tile_pool(name="stage", bufs=4) as stg,
            tc.tile_pool(name="hst", bufs=1) as hstp,
            tc.tile_pool(name="qtp", bufs=2) as qtp,
            tc.tile_pool(name="otp", bufs=2) as otp,
            tc.tile_pool(name="ep", bufs=4) as ep,
            tc.tile_pool(name="rt", bufs=2) as rtp,
            tc.tile_pool(name="ps_acc", bufs=4, space="PSUM") as ps_acc,
            tc.tile_pool(name="ps_r", bufs=1, space="PSUM") as ps_r,
            tc.tile_pool(name="ps_sc", bufs=2, space="PSUM") as ps_sc,
            tc.tile_pool(name="ps_tr", bufs=1, space="PSUM") as ps_tr,
            tc.tile_pool(name="dram", bufs=1, space="DRAM") as dram,
        ):
            # ---------------- persistent SBUF tensors ----------------
            wq_bf = pp.tile([128, DC, QCOLS], BF16, tag="wq")
            wk_bf = pp.tile([128, DC, HD], BF16, tag="wk")
            wv_bf = pp.tile([128, DC, HD], BF16, tag="wv")
            wo_bf = pp.tile([128, DC, QCOLS], BF16, tag="wo")
            cos_sb = pp.tile([HD, S], BF16, tag="cos")
            sin_sb = pp.tile([HD, S], BF16, tag="sin")
            kT_bf = pp.tile([HD, S], BF16, tag="kt")
            v_bf = pp.tile([128, S // 128, HD], BF16, tag="v")
            ident = pp.tile([128, 128], BF16, tag="id")
            ones_col = pp.tile([128, 1], BF16, tag="onc")
            ones_row = pp.tile([1, 128], BF16, tag="onr")

            attn_in = [
                [
                    dram.tile(
                        [HD, CHUNK], BF16, tag=f"ain{j}_{h}", name=f"ain{j}_{h}"
                    )
                    for h in range(NH)
                ]
                for j in range(NCHUNK)
            ]
            attn_all = [
                [
                    dram.tile(
                        [NCORES * HD, CHUNK], BF16, tag=f"aall{j}_{h}",
                        addr_space="Shared", name=f"aall{j}_{h}",
                    )
                    for h in range(NH)
                ]
                for j in range(NCHUNK)
            ]

            # ---------------- constants (cheap, first) ----------------
            make_identity(nc, ident[:])
            nc.gpsimd.memset(ones_col[:], 1.0)
            nc.gpsimd.memset(ones_row[:], 1.0)
            nc.sync.dma_start(out=cos_sb[:], in_=cos_d)
            nc.sync.dma_start(out=sin_sb[:], in_=sin_d)

            # ---------------- phase helpers ----------------
            def load_weight(dst_bf, src_d, width, eng, eng2=None):
                # two d-chunks per DMA (512KB when width=512) for DMA efficiency
                sview = src_d.rearrange("(i p) w -> p i w", p=128)
                for i in range(0, DC, 2):
                    ws = stg.tile([128, 1024], F32, tag="stage")
                    wsv = ws[:].rearrange("p (i w) -> p i w", i=2)
                    nc.scalar.dma_start(
                        out=wsv[:, :, :width], in_=sview[:, i : i + 2, :]
                    )
                    e1 = eng if (eng2 is None or (i // 2) % 2 == 0) else eng2
                    e1(out=dst_bf[:, i, :], in_=wsv[:, 0, :width])
                    e1(out=dst_bf[:, i + 1, :], in_=wsv[:, 1, :width])

            def build_hsT(j, hsT):
                """DMA hs rows, convert to bf16, transpose with PE into hsT."""
                s0 = j * CHUNK
                for dp in range(4):  # 4 column pieces of 1024
                    for ss in range(CHUNK // 128):
                        r0 = s0 + 128 * ss
                        hstg = stg.tile([128, 1024], F32, tag="stage")
                        nc.sync.dma_start(
                            out=hstg[:],
                            in_=hs_d[r0 : r0 + 128, 1024 * dp : 1024 * (dp + 1)],
                        )
                        hsb = stg.tile([128, 1024], BF16, tag="hsb", bufs=2)
                        nc.vector.tensor_copy(out=hsb[:], in_=hstg[:])
                        for u in range(2):  # two groups of 4 transposes
                            tr = ps_tr.tile([128, 512], BF16, tag="tr")
                            for t in range(4):
                                kk = 4 * u + t
                                nc.tensor.transpose(
                                    tr[:, 128 * t : 128 * (t + 1)],
                                    hsb[:, 128 * kk : 128 * (kk + 1)],
                                    ident[:],
                                )
                            i0 = 8 * dp + 4 * u
                            nc.any.tensor_copy(
                                out=hsT[:, i0 : i0 + 4, 128 * ss : 128 * (ss + 1)],
                                in_=tr[:].rearrange("p (a b) -> p a b", a=4),
                            )

            def rope(psrc, dst_ap, sl):
                t1 = rtp.tile([128, CHUNK], F32, tag="rt")
                t2 = rtp.tile([128, CHUNK], F32, tag="rt")
                nc.vector.tensor_tensor(
                    out=t1[0:64, :], in0=psrc[64:128, :], in1=sin_sb[0:64, sl], op=Alu.mult
                )
                nc.vector.tensor_tensor(
                    out=t1[64:128, :], in0=psrc[0:64, :], in1=sin_sb[64:128, sl], op=Alu.mult
                )
                nc.vector.tensor_tensor(
                    out=t2[:], in0=psrc[:], in1=cos_sb[:, sl], op=Alu.mult
                )
                nc.vector.tensor_tensor(out=dst_ap, in0=t1[:], in1=t2[:], op=Alu.add)

            def projections(j, hsT, qT):
                s0 = j * CHUNK
                sl = bass.ds(s0, CHUNK)
                for h in range(NH):
                    psq = ps_acc.tile([128, 512], F32, tag="acc")
                    for i in range(DC):
                        nc.tensor.matmul(
                            psq[:],
                            lhsT=wq_bf[:, i, HD * h : HD * (h + 1)],
                            rhs=hsT[:, i, :],
                            start=(i == 0),
                            stop=(i == DC - 1),
                        )
                    rope(psq[:], qT[:, h, :], sl)
                psk = ps_acc.tile([128, 512], F32, tag="acc")
                for i in range(DC):
                    nc.tensor.matmul(
                        psk[:],
                        lhsT=wk_bf[:, i, :],
                        rhs=hsT[:, i, :],
                        start=(i == 0),
                        stop=(i == DC - 1),
                    )
                rope(psk[:], kT_bf[:, sl], sl)
                # v computed as vT (stationary = wv reused 512-wide), then
                # PE-transposed back to [s, hd] for the AV matmuls: avoids 512
                # LDWEIGHTS-bound 128-wide matmuls.
                psvT = ps_acc.tile([128, 512], F32, tag="acc")
                for i in range(DC):
                    nc.tensor.matmul(
                        psvT[:],
                        lhsT=wv_bf[:, i, :],
                        rhs=hsT[:, i, :],
                        start=(i == 0),
                        stop=(i == DC - 1),
                    )
                vT_sb = ep.tile([128, 512], BF16, tag="vts", bufs=2)
                nc.scalar.copy(out=vT_sb[:], in_=psvT[:])
                psv2 = ps_acc.tile([128, 4, 128], BF16, tag="acc")
                for ss in range(CHUNK // 128):
                    nc.tensor.transpose(
                        psv2[:, ss, :],
                        vT_sb[:, 128 * ss : 128 * (ss + 1)],
                        ident[:],
                    )
                nc.any.tensor_copy(
                    out=v_bf[:, (CHUNK // 128) * j : (CHUNK // 128) * (j + 1), :],
                    in_=psv2[:],
                )

            def attention(j, qT):
                nk = 4 * (j + 1)  # causal: key tiles 0..nk-1

                def score_block(h, kcs):
                    es = []
                    for kc in kcs:
                        pss = ps_sc.tile([128, 512], F32, tag="sc")
                        nc.tensor.matmul(
                            pss[:],
                            lhsT=kT_bf[:, 128 * kc : 128 * (kc + 1)],
                            rhs=qT[:, h, :],
                            start=True,
                            stop=True,
                        )
                        e = ep.tile([128, CHUNK], BF16, tag="e", bufs=8)
                        nc.scalar.activation(
                            out=e[:], in_=pss[:], func=Act.Exp, scale=SCALE
                        )
                        if kc >= 4 * j:
                            # causal mask in-place on the idle gpsimd:
                            # keep where q_rel - k_rel - 128*(kc-4j) >= 0
                            nc.gpsimd.affine_select(
                                out=e[:],
                                in_=e[:],
                                compare_op=Alu.is_ge,
                                fill=0.0,
                                base=-128 * (kc - 4 * j),
                                channel_multiplier=-1,
                                pattern=[[1, CHUNK]],
                            )
                        es.append(e)
                    return es

                def av_block(pso, es, kcs):
                    for e, kc in zip(es, kcs):
                        nc.tensor.matmul(
                            pso[:],
                            lhsT=v_bf[:, kc, :],
                            rhs=e[:],
                            start=(kc == 0),
                            stop=(kc == nk - 1),
                        )

                def racc_block(racc, es, kcs):
                    for e, kc in zip(es, kcs):
                        if kc == 0:
                            nc.vector.tensor_copy(out=racc[:], in_=e[:])
                        else:
                            nc.vector.tensor_tensor(
                                out=racc[:], in0=racc[:], in1=e[:], op=Alu.add
                            )

                # two heads interleaved: one head's score/AV matmuls hide the
                # other head's exp+mask latency on ACT/gpsimd
                def emit_epilogues(pairs):
                    for h, pso, racc in pairs:
                        # partition reduce 128 -> 1 with a single ones-matmul
                        rbf = ep.tile([128, 512], BF16, tag="rbf", bufs=2)
                        nc.scalar.copy(out=rbf[:], in_=racc[:])
                        psr = ps_r.tile([1, 512], F32, tag="r")
                        nc.tensor.matmul(
                            psr[:], lhsT=ones_col[:], rhs=rbf[:],
                            start=True, stop=True,
                        )
                        rc = ep.tile([1, 512], BF16, tag="rc", bufs=2)
                        with nc.allow_low_precision(
                            reason="1/rowsum bcast; bf16 fine for softmax norm"
                        ):
                            nc.vector.reciprocal(rc[:], psr[:])
                        psb = ps_r.tile([128, 512], F32, tag="r")
                        nc.tensor.matmul(
                            psb[:], lhsT=ones_row[:], rhs=rc[:],
                            start=True, stop=True,
                        )
                        sbb = ep.tile([128, 512], F32, tag="os", bufs=3)
                        nc.scalar.copy(out=sbb[:], in_=psb[:])
                        ao = ep.tile([128, CHUNK], BF16, tag="ao", bufs=2)
                        nc.vector.tensor_tensor(
                            out=ao[:], in0=pso[:], in1=sbb[:], op=Alu.mult
                        )
                        nc.sync.dma_start(out=attn_in[j][h][:, :], in_=ao[:])
                        nc.gpsimd.collective_compute(
                            "AllGather",
                            Alu.bypass,
                            replica_groups=[list(range(NCORES))],
                            ins=[attn_in[j][h].opt()],
                            outs=[attn_all[j][h].opt()],
                        )

                pending = None
                for hp in range(NH // 2):
                    h0, h1 = 2 * hp, 2 * hp + 1
                    pso0 = ps_acc.tile([128, 512], F32, tag="acc")
                    pso1 = ps_acc.tile([128, 512], F32, tag="acc")
                    racc0 = ep.tile([128, 512], F32, tag="racc", bufs=2)
                    racc1 = ep.tile([128, 512], F32, tag="racc2", bufs=2)
                    for kb in range(0, nk, 4):
                        kcs = list(range(kb, min(kb + 4, nk)))
                        es0 = score_block(h0, kcs)
                        es1 = score_block(h1, kcs)
                        av_block(pso0, es0, kcs)
                        av_block(pso1, es1, kcs)
                        racc_block(racc0, es0, kcs)
                        racc_block(racc1, es1, kcs)
                    # defer this pair's epilogue until after the NEXT pair's
                    # kc loop: the reduce/recip/bcast cross-engine chain then
                    # resolves behind real matmul work instead of stalling PE
                    if pending is not None:
                        emit_epilogues(pending)
                    pending = [(h0, pso0, racc0), (h1, pso1, racc1)]
                emit_epilogues(pending)

            def oproj(j):
                aviews = [
                    attn_all[j][h][:].rearrange("(r p) s -> p r s", p=128)
                    for h in range(NH)
                ]
                for ss in range(CHUNK // 128):
                    ot = otp.tile([128, DC, 128], BF16, tag="ot")
                    otv = ot[:].rearrange("p (r h) f -> p r h f", h=NH)
                    for h in range(NH):
                        nc.scalar.dma_start(
                            out=otv[:, :, h, :],
                            in_=aviews[h][:, :, 128 * ss : 128 * (ss + 1)],
                        )
                    pso2 = ps_acc.tile([128, 512], F32, tag="acc")
                    order = [4 * r + h for h in range(NH) for r in range(NCORES)]
                    for n, g in enumerate(order):
                        nc.tensor.matmul(
                            pso2[:],
                            lhsT=ot[:, g, :],
                            rhs=wo_bf[:, g, :],
                            start=(n == 0),
                            stop=(n == DC - 1),
                        )
                    osb = ep.tile([128, 512], F32, tag="os", bufs=3)
                    nc.any.tensor_copy(out=osb[:], in_=pso2[:])
                    r0 = j * CHUNK + 128 * ss
                    nc.sync.dma_start(out=out_d[r0 : r0 + 128, :], in_=osb[:])

            # ---------------- schedule ----------------
            # hs chunk 0 first so the PE starts transposing immediately;
            # q/k/v weights next (needed by chunk-0 projections); wo deferred.
            # hs chunk 0 first (feeds PE transposes immediately), then the
            # weights in the order projections consume them; wo deferred.
            hsT0 = hstp.tile([128, DC, CHUNK], BF16, tag="hsT")
            build_hsT(0, hsT0)
            load_weight(wq_bf, wq_d, QCOLS, nc.scalar.copy, nc.vector.tensor_copy)
            load_weight(wk_bf, wk_d, HD, nc.vector.tensor_copy)
            load_weight(wv_bf, wv_d, HD, nc.vector.tensor_copy)

            qT0 = qtp.tile([HD, NH, CHUNK], BF16, tag="qT")
            projections(0, hsT0, qT0)
            attention(0, qT0)

            for j in range(1, NCHUNK):
                hsT = hstp.tile([128, DC, CHUNK], BF16, tag="hsT")
                build_hsT(j, hsT)
                qT = qtp.tile([HD, NH, CHUNK], BF16, tag="qT")
                projections(j, hsT, qT)
                if j == 1:
                    load_weight(wo_bf, wo_d, QCOLS, nc.vector.tensor_copy, nc.scalar.copy)
                attention(j, qT)
                oproj(j - 1)
            oproj(NCHUNK - 1)

    nc.finalize()
    return nc


def _get_graph():
    if "nc" not in _CACHED:
        _CACHED["nc"] = _build_graph()
    return _CACHED["nc"]


def _rope_tables(position_ids):
    pos = np.asarray(position_ids).reshape(-1).astype(np.float64)  # [S]
    inv_freq = 1.0 / (10000.0 ** (np.arange(0, HD, 2, dtype=np.float64) / HD))  # [64]
    freqs = pos[:, None] * inv_freq[None, :]  # [S, 64]
    emb = np.concatenate([freqs, freqs], axis=-1)  # [S, HD]
    cos_t = np.cos(emb).T.astype(np.float32)  # [HD, S]
    sin_t = np.sin(emb).T.astype(np.float32)
    sin_signed = sin_t.copy()
    sin_signed[: HD // 2] *= -1.0
    bf = ml_dtypes.bfloat16
    return (
        np.ascontiguousarray(cos_t.astype(bf)),
        np.ascontiguousarray(sin_signed.astype(bf)),
    )


def kernel(hidden_states, wq, wk, wv, wo, position_ids, _trace=False):
    hs = np.ascontiguousarray(np.asarray(hidden_states, np.float32).reshape(S, D))
    wq = np.asarray(wq, np.float32)
    wk = np.asarray(wk, np.float32)
    wv = np.asarray(wv, np.float32)
    wo = np.asarray(wo, np.float32)
    cos_t, sin_t = _rope_tables(position_ids)

    in_maps = []
    for c in range(NCORES):
        in_maps.append(
            {
                "hs": hs,
                "wq": np.ascontiguousarray(wq[:, QCOLS * c : QCOLS * (c + 1)]),
                "wk": np.ascontiguousarray(wk[:, HD * c : HD * (c + 1)]),
                "wv": np.ascontiguousarray(wv[:, HD * c : HD * (c + 1)]),
                "wo": np.ascontiguousarray(wo[:, QCOLS * c : QCOLS * (c + 1)]),
                "cos": cos_t,
                "sin": sin_t,
            }
        )

    nc = _get_graph()
    res = run_bass_kernel_spmd(
        nc, in_maps, core_ids=list(range(NCORES)), trace=_trace
    )
    outs = [np.asarray(res.results[c]["out"]) for c in range(NCORES)]
    full = np.concatenate(outs, axis=1).reshape(1, S, D).astype(np.float32)
    if _trace:
        kernel.last_results = res
    return full
